# revision 16
# baseline (speedup 1.0000x reference)
"""Causal multi-head attention (B=4, S=2048, H=2048, NH=16) on 8 TRN2 NeuronCores.

Strategy (tensor-parallel over heads + all-to-all reshard), v2:
  - Each core owns 2 heads. Host slices W_attn/b_attn per core, casts to
    bf16, and relayouts x^T / weights so every SBUF load is one big DMA
    ([128, kc, cols] layouts; fp32 accumulation happens in PSUM).
  - Phase A (per batch, 24 emission units): QKV projection from x^T
    tiles produces Q^T, K^T ([head_dim, tokens]) and V ([tokens,
    head_dim] with interleaved ones columns carrying a free softmax
    denominator through the PV matmul).
  - Phase B (per batch, per head): scores^T = K^T.T @ Q^T on causal
    blocks only, two k-blocks per 2-bank PSUM tile so exp runs as one
    ScalarE ACTIVATE per pair; P^T tiles are PV stationary operands,
    rhs = [V | ones]. Normalize with per-row reciprocal on VectorE.
    One AllToAll per batch reshards head-parallel -> token-parallel.
  - Phase C: exact output projection for this core's 256-token slice of
    each batch; b_proj enters via a rank-1 ones matmul.
  - Software-pipelined emission: B(b) interleaves one unit of A(b+1)
    (or C(b-3)) between each score group and its PV group, so the PE
    stream never waits on ScalarE exp or on a collective. DMA triggers
    live only on the sync/gpsimd queues; ScalarE does exp exclusively.

Self-contained: hardcodes all shapes; no file reads.
"""

import itertools

import numpy as np
import ml_dtypes

_SENT = object()

import concourse.bacc as bacc
import concourse.tile as tile
import concourse.mybir as mybir
from concourse import bass_utils

BF16 = mybir.dt.bfloat16
F32 = mybir.dt.float32
AF = mybir.ActivationFunctionType

N_CORES = 8
B = 4
S = 2048
H = 2048
NH = 16
HD = 128
HPC = NH // N_CORES          # heads per core = 2
TOK = B * S                  # 8192
KCH = H // 128               # 16 hidden chunks
SC = 512                     # token chunk for projections / q-chunks
TPB_CH = S // SC             # 4 token chunks per batch
QB = S // 128                # 16 q/kv blocks per batch
SCALE = 1.0 / float(np.sqrt(HD))
VSTRIDE = 2 * (HD + 1)       # V storage: per tokblock [Vh0|1|Vh1|1]
TPB = S // N_CORES           # 256 tokens per core per batch after A2A

_CACHE: dict = {}
LAST_RESULT = None


def _build():
    nc = bacc.Bacc("TRN2", target_bir_lowering=False, debug=False,
                   num_devices=N_CORES)
    # Host-relayouted inputs: leading dim 128 = SBUF partition.
    xT = nc.dram_tensor("xT", [128, KCH, TOK], BF16, kind="ExternalInput")
    wqkv = nc.dram_tensor("wqkv", [128, KCH, 6 * HD], BF16,
                          kind="ExternalInput")
    wproj = nc.dram_tensor("wproj", [128, KCH, H], BF16,
                           kind="ExternalInput")
    bqkv = nc.dram_tensor("bqkv", [1, 6 * HD], BF16, kind="ExternalInput")
    bqk_t = nc.dram_tensor("bqk_t", [128, 4], F32, kind="ExternalInput")
    bproj = nc.dram_tensor("bproj", [1, H], BF16, kind="ExternalInput")
    mask = nc.dram_tensor("mask", [128, 128], BF16, kind="ExternalInput")
    out = nc.dram_tensor("out", [B * TPB, H], F32, kind="ExternalOutput")

    with tile.TileContext(nc) as tc:
        with (
            tc.tile_pool(name="const", bufs=1) as constp,
            tc.tile_pool(name="qkp", bufs=8) as qkp,
            tc.tile_pool(name="dram", bufs=1, space="DRAM") as dram,
            tc.tile_pool(name="xTp", bufs=3) as xTp,
            tc.tile_pool(name="psAC", bufs=2, space="PSUM") as psAC,
            tc.tile_pool(name="psS", bufs=2, space="PSUM") as psS,
            tc.tile_pool(name="psPV", bufs=2, space="PSUM") as psPV,
            tc.tile_pool(name="ptP", bufs=9) as ptP,
            tc.tile_pool(name="an4P", bufs=2) as an4P,
            tc.tile_pool(name="recP", bufs=4) as recP,
            tc.tile_pool(name="atP", bufs=4) as atP,
            tc.tile_pool(name="outP", bufs=2) as outP,
        ):
            # ---- resident weights / consts -------------------------------
            wq_t = constp.tile([128, KCH * 6 * HD], BF16, name="wq_t")
            nc.sync.dma_start(wq_t[:, 0:8 * 6 * HD], wqkv[:, 0:8, :])

            # First x chunk right behind the first weight half.
            xt_tiles = [None] * (2 * TPB_CH * B)   # (t, half) -> tile

            def load_x(t, half):
                xtile = xTp.tile([128, 8 * SC], BF16, name="xt")
                nc.sync.dma_start(
                    xtile[:],
                    xT[:, half * 8:(half + 1) * 8, t * SC:(t + 1) * SC])
                xt_tiles[2 * t + half] = xtile

            load_x(0, 0)
            nc.sync.dma_start(wq_t[:, 8 * 6 * HD:], wqkv[:, 8:16, :])
            load_x(0, 1)

            mask_sb = constp.tile([128, 128], BF16, name="mask_sb")
            nc.gpsimd.dma_start(mask_sb[:], mask[:])
            ones_sb = constp.tile([1, 128], BF16, name="ones_sb")
            nc.vector.memset(ones_sb[:], 1.0)
            bqkv_sb = constp.tile([1, 6 * HD], BF16, name="bqkv_sb")
            nc.gpsimd.dma_start(bqkv_sb[:], bqkv[:])
            bqkt_sb = constp.tile([128, 4], F32, name="bqkt_sb")
            nc.gpsimd.dma_start(bqkt_sb[:], bqk_t[:])
            bproj_sb = constp.tile([1, H], BF16, name="bproj_sb")
            nc.gpsimd.dma_start(bproj_sb[:], bproj[:])

            # V stores: 2 persistent slots; ones columns memset once.
            vst = [constp.tile([128, QB * VSTRIDE], BF16, name=f"vst{i}")
                   for i in range(2)]
            nc.vector.memset(vst[0][:], 1.0)
            nc.vector.memset(vst[1][:], 1.0)

            # W_proj resident, loaded late on gpsimd (first used in B3).
            wp_t = constp.tile([128, KCH * H], BF16, name="wp_t")

            def load_wproj(c):
                nc.gpsimd.dma_start(
                    wp_t[:, c * 4 * H:(c + 1) * 4 * H],
                    wproj[:, c * 4:(c + 1) * 4, :])

            qk_store = [None] * B
            a2a_in = [dram.tile([S, HPC * HD], BF16, name=f"cc_in{b}")
                      for b in range(B)]
            a2a_out = [dram.tile([S, HPC * HD], BF16, name=f"cc_out{b}")
                       for b in range(B)]
            at_w = [None] * B

            # ---- phase A as a micro-step generator -----------------------
            # Yields roughly every 3-4 matmuls so phase B can interleave
            # filler work between score pairs at sub-microsecond grain.
            def phase_a_steps(b):
                qk_store[b] = [qkp.tile([128, S], BF16, name="qkt")
                               for _ in range(4)]
                vslot = vst[b % 2]
                for tloc in range(TPB_CH):
                    t = b * TPB_CH + tloc
                    for ob in range(4):      # q_h0, q_h1, k_h0, k_h1
                        ps = psAC.tile([128, SC], F32, name="psa")
                        for kc in range(KCH):
                            xth = xt_tiles[2 * t + kc // 8]
                            nc.tensor.matmul(
                                ps[:],
                                wq_t[:, kc * 6 * HD + ob * 128:
                                     kc * 6 * HD + (ob + 1) * 128],
                                xth[:, (kc % 8) * SC:(kc % 8 + 1) * SC],
                                start=(kc == 0), stop=(kc == KCH - 1))
                            if kc % 4 == 3:
                                yield
                        nc.vector.tensor_scalar_add(
                            qk_store[b][ob][:, tloc * SC:(tloc + 1) * SC],
                            ps[:], bqkt_sb[:, ob:ob + 1])
                    for tbp in range(2):     # V blocks, natural layout
                        # prefetch next tchunk (crossing into the next
                        # batch at tloc==3; slot WAR throttles timing)
                        if t + 1 < B * TPB_CH:
                            load_x(t + 1, tbp)
                        for tb in (2 * tbp, 2 * tbp + 1):
                            psw = psAC.tile([128, SC], F32, name="psa")
                            ps = psw[:, 0:2 * HD]
                            for kc in range(KCH):
                                xth = xt_tiles[2 * t + kc // 8]
                                nc.tensor.matmul(
                                    ps,
                                    xth[:, (kc % 8) * SC + tb * 128:
                                        (kc % 8) * SC + (tb + 1) * 128],
                                    wq_t[:, kc * 6 * HD + 4 * HD:
                                         kc * 6 * HD + 6 * HD],
                                    start=(kc == 0), stop=False)
                                if kc % 8 == 7:
                                    yield
                            nc.tensor.matmul(ps, ones_sb[:],
                                             bqkv_sb[:, 4 * HD:6 * HD],
                                             start=False, stop=True)
                            base = (tloc * 4 + tb) * VSTRIDE
                            nc.vector.tensor_copy(
                                vslot[:, base:base + HD], ps[:, 0:HD])
                            nc.vector.tensor_copy(
                                vslot[:, base + HD + 1:
                                      base + 2 * HD + 1],
                                ps[:, HD:2 * HD])
                        yield

            # ---- phase C as a unit generator -----------------------------
            def phase_c_transposes(b):
                """Two wide DMA-transposes bring the received buffer in as
                a^T: partition = hidden-within-head-half, free =
                shard*TPB + token."""
                at_w[b] = []
                for half in range(2):
                    atile = atP.tile([128, S], BF16, name="at")
                    nc.sync.dma_start(
                        atile[:],
                        a2a_out[b][:, half * 128:(half + 1) * 128],
                        transpose=True)
                    at_w[b].append(atile)

            def phase_c_steps(b):
                """Micro-step generator: output projection for this core's
                token slice of batch b."""
                for oc in range(4):
                    for tb in range(TPB // 128):
                        ps = psAC.tile([128, SC], F32, name="psa")
                        for hc in range(KCH):
                            nc.tensor.matmul(
                                ps[:],
                                at_w[b][hc % 2][:, (hc // 2) * TPB
                                                + tb * 128:
                                                (hc // 2) * TPB
                                                + (tb + 1) * 128],
                                wp_t[:, hc * H + oc * SC:
                                     hc * H + (oc + 1) * SC],
                                start=(hc == 0), stop=False)
                            if hc % 4 == 3:
                                yield
                        nc.tensor.matmul(
                            ps[:], ones_sb[:],
                            bproj_sb[:, oc * SC:(oc + 1) * SC],
                            start=False, stop=True)
                        ot = outP.tile([128, SC], F32, name="ot")
                        nc.vector.tensor_copy(ot[:], ps[:])
                        nc.gpsimd.dma_start(
                            out[b * TPB + tb * 128:
                                b * TPB + (tb + 1) * 128,
                                oc * SC:(oc + 1) * SC],
                            ot[:])

            # ---- phase B with interleaved filler units -------------------
            def phase_b(b, filler_it):
                """Attention for batch b (both heads) + its AllToAll.
                Pulls filler micro-steps between score pairs so the PE
                stream stays fed while psS ping-pongs with ScalarE exp."""
                vslot = vst[b % 2]

                def fill(n):
                    for _ in range(n):
                        if next(filler_it, _SENT) is _SENT:
                            break

                for h in range(HPC):
                    qt = qk_store[b][h]
                    kt = qk_store[b][2 + h]
                    for qc in range(4):
                        pts = []
                        for pr in range(2 * (qc + 1)):
                            ps2 = psS.tile([128, 2 * SC], F32, name="pss")
                            col0s = []
                            for j in range(2):
                                kb = 2 * pr + j
                                col0 = max(0, kb * 128 - qc * SC)
                                col0s.append(col0)
                                nc.tensor.matmul(
                                    ps2[:, j * SC + col0:(j + 1) * SC],
                                    kt[:, kb * 128:(kb + 1) * 128],
                                    qt[:, qc * SC + col0:(qc + 1) * SC],
                                    start=True, stop=True)
                            pt2 = ptP.tile([128, 2 * SC], BF16, name="pt")
                            if col0s[0] == col0s[1]:
                                # one ACT covers both written halves
                                nc.scalar.activation(
                                    pt2[:, col0s[0]:2 * SC],
                                    ps2[:, col0s[0]:2 * SC],
                                    AF.Exp, scale=SCALE)
                            else:
                                for j in range(2):
                                    nc.scalar.activation(
                                        pt2[:, j * SC + col0s[j]:(j + 1) * SC],
                                        ps2[:, j * SC + col0s[j]:(j + 1) * SC],
                                        AF.Exp, scale=SCALE)
                            for j in range(2):
                                kb = 2 * pr + j
                                if kb >= 4 * qc:
                                    col0 = max(0, kb * 128 - qc * SC)
                                    nc.vector.tensor_mul(
                                        pt2[:, j * SC + col0:
                                            j * SC + col0 + 128],
                                        pt2[:, j * SC + col0:
                                            j * SC + col0 + 128],
                                        mask_sb[:])
                            pts.append(pt2)
                            fill(1)
                        an4 = an4P.tile([128, 4 * HD], BF16, name="an4")
                        for qb in range(4):
                            qg = qc * 4 + qb
                            po = psPV.tile([128, SC], F32,
                                           name="ppv")[:, 0:HD + 1]
                            for kb in range(qg + 1):
                                pr, j = divmod(kb, 2)
                                vbase = kb * VSTRIDE + h * (HD + 1)
                                nc.tensor.matmul(
                                    po[:],
                                    pts[pr][:, j * SC + qb * 128:
                                            j * SC + (qb + 1) * 128],
                                    vslot[:, vbase:vbase + HD + 1],
                                    start=(kb == 0), stop=(kb == qg))
                            rec = recP.tile([128, 1], F32, name="rec")
                            nc.vector.reciprocal(rec[:], po[:, HD:HD + 1])
                            nc.vector.tensor_scalar_mul(
                                an4[:, qb * 128:(qb + 1) * 128],
                                po[:, 0:HD], rec[:])
                            fill(1)
                        nc.sync.dma_start(
                            a2a_in[b][qc * SC:(qc + 1) * SC,
                                      h * HD:(h + 1) * HD]
                            .rearrange("(qb q) c -> q qb c", qb=4),
                            an4[:])
                nc.gpsimd.collective_compute(
                    "AllToAll",
                    mybir.AluOpType.bypass,
                    replica_groups=[list(range(N_CORES))],
                    ins=[a2a_in[b].opt()],
                    outs=[a2a_out[b].opt()],
                )

            def drain(it):
                for _ in it:
                    pass

            # ---- software-pipelined emission -----------------------------
            # A0(+wp spread) [B0*A1] A1rest [B1*A2] A2rest [B2*(C0,A3)]
            # A3rest [B3*C1] C2 C3.  Filler micro-steps keep the PE fed
            # inside B phases; wproj streams in during A0 so it never
            # contends with a collective.
            i = 0
            for _ in phase_a_steps(0):
                i += 1
                if i % 26 == 0 and i // 26 <= 4:
                    load_wproj(i // 26 - 1)
            a1 = phase_a_steps(1)
            phase_b(0, a1)
            drain(a1)
            a2 = phase_a_steps(2)
            phase_b(1, a2)
            drain(a2)
            phase_c_transposes(0)
            c0a3 = itertools.chain(phase_c_steps(0), phase_a_steps(3))
            phase_b(2, c0a3)
            drain(c0a3)
            phase_c_transposes(1)
            phase_c_transposes(2)
            c12 = itertools.chain(phase_c_steps(1), phase_c_steps(2))
            phase_b(3, c12)
            drain(c12)
            phase_c_transposes(3)
            drain(phase_c_steps(3))

    nc.compile()
    return nc


def _get_nc():
    if "nc" not in _CACHE:
        _CACHE["nc"] = _build()
    return _CACHE["nc"]


def kernel(hidden_states, W_attn, b_attn, W_proj, b_proj):
    global LAST_RESULT
    bf = ml_dtypes.bfloat16
    x = np.asarray(hidden_states, dtype=np.float32).reshape(TOK, H)
    xb = x.astype(bf)
    # [128, kc, t] layout: xT3[p, kc, t] = x[t, kc*128+p]
    xT3 = np.ascontiguousarray(
        xb.reshape(TOK, KCH, 128).transpose(2, 1, 0))
    Wa = np.asarray(W_attn, dtype=np.float32)
    ba = np.asarray(b_attn, dtype=np.float32)
    Wp = np.asarray(W_proj, dtype=np.float32).astype(bf)
    wp3 = np.ascontiguousarray(Wp.reshape(KCH, 128, H).transpose(1, 0, 2))
    bp = np.asarray(b_proj, dtype=np.float32).reshape(1, H).astype(bf)
    mask = np.triu(np.ones((128, 128), dtype=np.float32)).astype(bf)

    in_maps = []
    for c in range(N_CORES):
        h0 = c * HPC
        cols = []
        for part in range(3):          # q, k, v feature slices
            cols.append(np.arange(part * H + h0 * HD,
                                  part * H + (h0 + HPC) * HD))
        cols = np.concatenate(cols)    # 768 column indices
        wq = Wa[:, cols].astype(bf)
        wq3 = np.ascontiguousarray(
            wq.reshape(KCH, 128, 6 * HD).transpose(1, 0, 2))
        bq = ba[cols].reshape(1, 6 * HD).astype(bf)
        # per-partition bias for the 4 Q^T/K^T feature blocks
        bqk_t = np.ascontiguousarray(
            ba[cols[:4 * 128]].reshape(4, 128).T).astype(np.float32)
        in_maps.append({
            "xT": xT3,
            "wqkv": wq3,
            "wproj": wp3,
            "bqkv": bq,
            "bqk_t": bqk_t,
            "bproj": bp,
            "mask": mask,
        })

    nc = _get_nc()
    res = bass_utils.run_bass_kernel_spmd(
        nc, in_maps, core_ids=list(range(N_CORES)))
    LAST_RESULT = res

    full = np.empty((B, S, H), dtype=np.float32)
    for c in range(N_CORES):
        r = res.results[c]["out"]
        for b in range(B):
            full[b, c * TPB:(c + 1) * TPB, :] = r[b * TPB:(b + 1) * TPB, :]
    return full


# revision 20
# speedup vs baseline: 1.0215x; 1.0215x over previous
"""Causal multi-head attention (B=4, S=2048, H=2048, NH=16) on 8 TRN2 NeuronCores.

Strategy (tensor-parallel over heads + all-to-all reshard), v2:
  - Each core owns 2 heads. Host slices W_attn/b_attn per core, casts to
    bf16, and relayouts x^T / weights so every SBUF load is one big DMA
    ([128, kc, cols] layouts; fp32 accumulation happens in PSUM).
  - Phase A (per batch, 24 emission units): QKV projection from x^T
    tiles produces Q^T, K^T ([head_dim, tokens]) and V ([tokens,
    head_dim] with interleaved ones columns carrying a free softmax
    denominator through the PV matmul).
  - Phase B (per batch, per head): scores^T = K^T.T @ Q^T on causal
    blocks only, two k-blocks per 2-bank PSUM tile so exp runs as one
    ScalarE ACTIVATE per pair; P^T tiles are PV stationary operands,
    rhs = [V | ones]. Normalize with per-row reciprocal on VectorE.
    One AllToAll per batch reshards head-parallel -> token-parallel.
  - Phase C: exact output projection for this core's 256-token slice of
    each batch; b_proj enters via a rank-1 ones matmul.
  - Software-pipelined emission: B(b) interleaves one unit of A(b+1)
    (or C(b-3)) between each score group and its PV group, so the PE
    stream never waits on ScalarE exp or on a collective. DMA triggers
    live only on the sync/gpsimd queues; ScalarE does exp exclusively.

Self-contained: hardcodes all shapes; no file reads.
"""

import numpy as np
import ml_dtypes

import concourse.bacc as bacc
import concourse.tile as tile
import concourse.mybir as mybir
from concourse import bass_utils

BF16 = mybir.dt.bfloat16
F32 = mybir.dt.float32
AF = mybir.ActivationFunctionType

N_CORES = 8
B = 4
S = 2048
H = 2048
NH = 16
HD = 128
HPC = NH // N_CORES          # heads per core = 2
TOK = B * S                  # 8192
KCH = H // 128               # 16 hidden chunks
SC = 512                     # token chunk for projections / q-chunks
TPB_CH = S // SC             # 4 token chunks per batch
QB = S // 128                # 16 q/kv blocks per batch
SCALE = 1.0 / float(np.sqrt(HD))
VSTRIDE = 2 * (HD + 1)       # V storage: per tokblock [Vh0|1|Vh1|1]
TPB = S // N_CORES           # 256 tokens per core per batch after A2A

_CACHE: dict = {}
LAST_RESULT = None


def _build():
    nc = bacc.Bacc("TRN2", target_bir_lowering=False, debug=False,
                   num_devices=N_CORES)
    # Host-relayouted inputs: leading dim 128 = SBUF partition.
    xT = nc.dram_tensor("xT", [128, KCH, TOK], BF16, kind="ExternalInput")
    wqkv = nc.dram_tensor("wqkv", [128, KCH, 6 * HD], BF16,
                          kind="ExternalInput")
    wproj = nc.dram_tensor("wproj", [128, KCH, H], BF16,
                           kind="ExternalInput")
    bqkv = nc.dram_tensor("bqkv", [1, 6 * HD], BF16, kind="ExternalInput")
    bqk_t = nc.dram_tensor("bqk_t", [128, 4], F32, kind="ExternalInput")
    bproj = nc.dram_tensor("bproj", [1, H], BF16, kind="ExternalInput")
    mask = nc.dram_tensor("mask", [128, 128], BF16, kind="ExternalInput")
    out = nc.dram_tensor("out", [B * TPB, H], F32, kind="ExternalOutput")

    with tile.TileContext(nc) as tc:
        with (
            tc.tile_pool(name="const", bufs=1) as constp,
            tc.tile_pool(name="qkp", bufs=8) as qkp,
            tc.tile_pool(name="dram", bufs=1, space="DRAM") as dram,
            tc.tile_pool(name="xTp", bufs=4) as xTp,
            tc.tile_pool(name="psAC", bufs=2, space="PSUM") as psAC,
            tc.tile_pool(name="psS", bufs=2, space="PSUM") as psS,
            tc.tile_pool(name="psPV", bufs=2, space="PSUM") as psPV,
            tc.tile_pool(name="ptP", bufs=9) as ptP,
            tc.tile_pool(name="an4P", bufs=1) as an4P,
            tc.tile_pool(name="recP", bufs=4) as recP,
            tc.tile_pool(name="atP", bufs=3) as atP,
            tc.tile_pool(name="outP", bufs=1) as outP,
        ):
            # ---- resident weights / consts -------------------------------
            wq_t = constp.tile([128, KCH * 6 * HD], BF16, name="wq_t")
            nc.sync.dma_start(wq_t[:, 0:8 * 6 * HD], wqkv[:, 0:8, :])

            # First x chunk right behind the first weight half.
            xt_tiles = [None] * (2 * TPB_CH * B)   # (t, half) -> tile

            def load_x(t, half):
                xtile = xTp.tile([128, 8 * SC], BF16, name="xt")
                nc.sync.dma_start(
                    xtile[:],
                    xT[:, half * 8:(half + 1) * 8, t * SC:(t + 1) * SC])
                xt_tiles[2 * t + half] = xtile

            load_x(0, 0)
            nc.sync.dma_start(wq_t[:, 8 * 6 * HD:], wqkv[:, 8:16, :])
            load_x(0, 1)

            mask_sb = constp.tile([128, 128], BF16, name="mask_sb")
            nc.gpsimd.dma_start(mask_sb[:], mask[:])
            ones_sb = constp.tile([1, 128], BF16, name="ones_sb")
            nc.vector.memset(ones_sb[:], 1.0)
            bqkv_sb = constp.tile([1, 6 * HD], BF16, name="bqkv_sb")
            nc.gpsimd.dma_start(bqkv_sb[:], bqkv[:])
            bqkt_sb = constp.tile([128, 4], F32, name="bqkt_sb")
            nc.gpsimd.dma_start(bqkt_sb[:], bqk_t[:])
            bproj_sb = constp.tile([1, H], BF16, name="bproj_sb")
            nc.gpsimd.dma_start(bproj_sb[:], bproj[:])

            # V stores: 2 persistent slots; ones columns memset once.
            vst = [constp.tile([128, QB * VSTRIDE], BF16, name=f"vst{i}")
                   for i in range(2)]
            nc.vector.memset(vst[0][:], 1.0)
            nc.vector.memset(vst[1][:], 1.0)

            # W_proj resident, loaded late on gpsimd (first used in B3).
            wp_t = constp.tile([128, KCH * H], BF16, name="wp_t")

            def load_wproj(c):
                nc.gpsimd.dma_start(
                    wp_t[:, c * 4 * H:(c + 1) * 4 * H],
                    wproj[:, c * 4:(c + 1) * 4, :])

            qk_store = [None] * B
            a2a_in = [dram.tile([S, HPC * HD], BF16, name=f"cc_in{b}")
                      for b in range(B)]
            a2a_out = [dram.tile([S, HPC * HD], BF16, name=f"cc_out{b}")
                       for b in range(B)]
            at_w = [None] * B

            # ---- phase A as a unit generator -----------------------------
            def phase_a_units(b):
                """Yield 24 emission units for batch b's QKV projection."""
                qk_store[b] = [qkp.tile([128, S], BF16, name="qkt")
                               for _ in range(4)]
                vslot = vst[b % 2]
                for tloc in range(TPB_CH):
                    t = b * TPB_CH + tloc
                    for ob in range(4):      # q_h0, q_h1, k_h0, k_h1
                        def u_qk(t=t, tloc=tloc, ob=ob):
                            ps = psAC.tile([128, SC], F32, name="psa")
                            for kc in range(KCH):
                                xth = xt_tiles[2 * t + kc // 8]
                                nc.tensor.matmul(
                                    ps[:],
                                    wq_t[:, kc * 6 * HD + ob * 128:
                                         kc * 6 * HD + (ob + 1) * 128],
                                    xth[:, (kc % 8) * SC:(kc % 8 + 1) * SC],
                                    start=(kc == 0), stop=(kc == KCH - 1))
                            nc.vector.tensor_scalar_add(
                                qk_store[b][ob][:, tloc * SC:(tloc + 1) * SC],
                                ps[:], bqkt_sb[:, ob:ob + 1])
                        yield u_qk
                    for tbp in range(2):     # V blocks, natural layout
                        def u_v(t=t, tloc=tloc, tbp=tbp):
                            # prefetch next tchunk (crossing into the next
                            # batch at tloc==3; slot WAR throttles timing)
                            if t + 1 < B * TPB_CH:
                                load_x(t + 1, tbp)
                            for tb in (2 * tbp, 2 * tbp + 1):
                                psw = psAC.tile([128, SC], F32, name="psa")
                                ps = psw[:, 0:2 * HD]
                                for kc in range(KCH):
                                    xth = xt_tiles[2 * t + kc // 8]
                                    nc.tensor.matmul(
                                        ps,
                                        xth[:, (kc % 8) * SC + tb * 128:
                                            (kc % 8) * SC + (tb + 1) * 128],
                                        wq_t[:, kc * 6 * HD + 4 * HD:
                                             kc * 6 * HD + 6 * HD],
                                        start=(kc == 0), stop=False)
                                nc.tensor.matmul(ps, ones_sb[:],
                                                 bqkv_sb[:, 4 * HD:6 * HD],
                                                 start=False, stop=True)
                                base = (tloc * 4 + tb) * VSTRIDE
                                nc.vector.tensor_copy(
                                    vslot[:, base:base + HD], ps[:, 0:HD])
                                nc.vector.tensor_copy(
                                    vslot[:, base + HD + 1:
                                          base + 2 * HD + 1],
                                    ps[:, HD:2 * HD])
                        yield u_v

            # ---- phase C as a unit generator -----------------------------
            def phase_c_transposes(b):
                """Two wide DMA-transposes bring the received buffer in as
                a^T: partition = hidden-within-head-half, free =
                shard*TPB + token."""
                at_w[b] = []
                for half in range(2):
                    atile = atP.tile([128, S], BF16, name="at")
                    nc.sync.dma_start(
                        atile[:],
                        a2a_out[b][:, half * 128:(half + 1) * 128],
                        transpose=True)
                    at_w[b].append(atile)

            def phase_c_units(b):
                """Yield 8 units: output projection for this core's token
                slice of batch b."""
                for oc in range(4):
                    for tb in range(TPB // 128):
                        def u_c(oc=oc, tb=tb):
                            ps = psAC.tile([128, SC], F32, name="psa")
                            for hc in range(KCH):
                                nc.tensor.matmul(
                                    ps[:],
                                    at_w[b][hc % 2][:, (hc // 2) * TPB
                                                    + tb * 128:
                                                    (hc // 2) * TPB
                                                    + (tb + 1) * 128],
                                    wp_t[:, hc * H + oc * SC:
                                         hc * H + (oc + 1) * SC],
                                    start=(hc == 0), stop=False)
                            nc.tensor.matmul(
                                ps[:], ones_sb[:],
                                bproj_sb[:, oc * SC:(oc + 1) * SC],
                                start=False, stop=True)
                            ot = outP.tile([128, SC], F32, name="ot")
                            nc.vector.tensor_copy(ot[:], ps[:])
                            nc.gpsimd.dma_start(
                                out[b * TPB + tb * 128:
                                    b * TPB + (tb + 1) * 128,
                                    oc * SC:(oc + 1) * SC],
                                ot[:])
                        yield u_c

            # ---- phase B with interleaved filler units -------------------
            def phase_b(b, fillers):
                """Attention for batch b (both heads) + its AllToAll.
                Runs one filler unit between each score group and its PV
                group so the PE stream stays fed while ScalarE exps."""
                vslot = vst[b % 2]
                for h in range(HPC):
                    qt = qk_store[b][h]
                    kt = qk_store[b][2 + h]
                    for qc in range(4):
                        pts = []
                        for pr in range(2 * (qc + 1)):
                            ps2 = psS.tile([128, 2 * SC], F32, name="pss")
                            col0s = []
                            for j in range(2):
                                kb = 2 * pr + j
                                col0 = max(0, kb * 128 - qc * SC)
                                col0s.append(col0)
                                nc.tensor.matmul(
                                    ps2[:, j * SC + col0:(j + 1) * SC],
                                    kt[:, kb * 128:(kb + 1) * 128],
                                    qt[:, qc * SC + col0:(qc + 1) * SC],
                                    start=True, stop=True)
                            pt2 = ptP.tile([128, 2 * SC], BF16, name="pt")
                            if col0s[0] == col0s[1]:
                                # one ACT covers both written halves
                                nc.scalar.activation(
                                    pt2[:, col0s[0]:2 * SC],
                                    ps2[:, col0s[0]:2 * SC],
                                    AF.Exp, scale=SCALE)
                            else:
                                for j in range(2):
                                    nc.scalar.activation(
                                        pt2[:, j * SC + col0s[j]:(j + 1) * SC],
                                        ps2[:, j * SC + col0s[j]:(j + 1) * SC],
                                        AF.Exp, scale=SCALE)
                            for j in range(2):
                                kb = 2 * pr + j
                                if kb >= 4 * qc:
                                    col0 = max(0, kb * 128 - qc * SC)
                                    nc.vector.tensor_mul(
                                        pt2[:, j * SC + col0:
                                            j * SC + col0 + 128],
                                        pt2[:, j * SC + col0:
                                            j * SC + col0 + 128],
                                        mask_sb[:])
                            pts.append(pt2)
                        if fillers:
                            fillers.pop(0)()
                        an4 = an4P.tile([128, 4 * HD], BF16, name="an4")
                        for qb in range(4):
                            qg = qc * 4 + qb
                            po = psPV.tile([128, SC], F32,
                                           name="ppv")[:, 0:HD + 1]
                            for kb in range(qg + 1):
                                pr, j = divmod(kb, 2)
                                vbase = kb * VSTRIDE + h * (HD + 1)
                                nc.tensor.matmul(
                                    po[:],
                                    pts[pr][:, j * SC + qb * 128:
                                            j * SC + (qb + 1) * 128],
                                    vslot[:, vbase:vbase + HD + 1],
                                    start=(kb == 0), stop=(kb == qg))
                            rec = recP.tile([128, 1], F32, name="rec")
                            nc.vector.reciprocal(rec[:], po[:, HD:HD + 1])
                            nc.vector.tensor_scalar_mul(
                                an4[:, qb * 128:(qb + 1) * 128],
                                po[:, 0:HD], rec[:])
                        nc.sync.dma_start(
                            a2a_in[b][qc * SC:(qc + 1) * SC,
                                      h * HD:(h + 1) * HD]
                            .rearrange("(qb q) c -> q qb c", qb=4),
                            an4[:])
                nc.gpsimd.collective_compute(
                    "AllToAll",
                    mybir.AluOpType.bypass,
                    replica_groups=[list(range(N_CORES))],
                    ins=[a2a_in[b].opt()],
                    outs=[a2a_out[b].opt()],
                )

            def run_units(units):
                for u in units:
                    u()

            # ---- software-pipelined emission -----------------------------
            # A0(+wp spread) [B0*A1] A1rest [B1*A2] A2rest [B2*C0] A3
            # [B3*C1] C2 C3.  Each A2A gets >=1 C phase of PE cover; each
            # C(b)'s transposes are emitted well before first use; wproj
            # streams in during A0 so it never contends with a collective.
            a0 = list(phase_a_units(0))
            for i, u in enumerate(a0):
                u()
                if i % 6 == 5:
                    load_wproj(i // 6)
            a1 = list(phase_a_units(1))
            phase_b(0, a1[:8])
            run_units(a1[8:])
            a2 = list(phase_a_units(2))
            phase_b(1, a2[:8])
            run_units(a2[8:])
            phase_c_transposes(0)
            c0 = list(phase_c_units(0))
            phase_b(2, c0)
            run_units(list(phase_a_units(3)))
            phase_c_transposes(1)
            c1 = list(phase_c_units(1))
            phase_b(3, c1)
            phase_c_transposes(2)
            phase_c_transposes(3)
            run_units(list(phase_c_units(2)))
            run_units(list(phase_c_units(3)))

    nc.compile()
    return nc


def _get_nc():
    if "nc" not in _CACHE:
        _CACHE["nc"] = _build()
    return _CACHE["nc"]


def kernel(hidden_states, W_attn, b_attn, W_proj, b_proj):
    global LAST_RESULT
    bf = ml_dtypes.bfloat16
    x = np.asarray(hidden_states, dtype=np.float32).reshape(TOK, H)
    xb = x.astype(bf)
    # [128, kc, t] layout: xT3[p, kc, t] = x[t, kc*128+p]
    xT3 = np.ascontiguousarray(
        xb.reshape(TOK, KCH, 128).transpose(2, 1, 0))
    Wa = np.asarray(W_attn, dtype=np.float32)
    ba = np.asarray(b_attn, dtype=np.float32)
    Wp = np.asarray(W_proj, dtype=np.float32).astype(bf)
    wp3 = np.ascontiguousarray(Wp.reshape(KCH, 128, H).transpose(1, 0, 2))
    bp = np.asarray(b_proj, dtype=np.float32).reshape(1, H).astype(bf)
    mask = np.triu(np.ones((128, 128), dtype=np.float32)).astype(bf)

    in_maps = []
    for c in range(N_CORES):
        h0 = c * HPC
        cols = []
        for part in range(3):          # q, k, v feature slices
            cols.append(np.arange(part * H + h0 * HD,
                                  part * H + (h0 + HPC) * HD))
        cols = np.concatenate(cols)    # 768 column indices
        wq = Wa[:, cols].astype(bf)
        wq3 = np.ascontiguousarray(
            wq.reshape(KCH, 128, 6 * HD).transpose(1, 0, 2))
        bq = ba[cols].reshape(1, 6 * HD).astype(bf)
        # per-partition bias for the 4 Q^T/K^T feature blocks
        bqk_t = np.ascontiguousarray(
            ba[cols[:4 * 128]].reshape(4, 128).T).astype(np.float32)
        in_maps.append({
            "xT": xT3,
            "wqkv": wq3,
            "wproj": wp3,
            "bqkv": bq,
            "bqk_t": bqk_t,
            "bproj": bp,
            "mask": mask,
        })

    nc = _get_nc()
    res = bass_utils.run_bass_kernel_spmd(
        nc, in_maps, core_ids=list(range(N_CORES)))
    LAST_RESULT = res

    full = np.empty((B, S, H), dtype=np.float32)
    for c in range(N_CORES):
        r = res.results[c]["out"]
        for b in range(B):
            full[b, c * TPB:(c + 1) * TPB, :] = r[b * TPB:(b + 1) * TPB, :]
    return full


# revision 21
# speedup vs baseline: 1.0309x; 1.0092x over previous
"""Causal multi-head attention (B=4, S=2048, H=2048, NH=16) on 8 TRN2 NeuronCores.

Strategy (tensor-parallel over heads + all-to-all reshard), v2:
  - Each core owns 2 heads. Host slices W_attn/b_attn per core, casts to
    bf16, and relayouts x^T / weights so every SBUF load is one big DMA
    ([128, kc, cols] layouts; fp32 accumulation happens in PSUM).
  - Phase A (per batch, 24 emission units): QKV projection from x^T
    tiles produces Q^T, K^T ([head_dim, tokens]) and V ([tokens,
    head_dim] with interleaved ones columns carrying a free softmax
    denominator through the PV matmul).
  - Phase B (per batch, per head): scores^T = K^T.T @ Q^T on causal
    blocks only, two k-blocks per 2-bank PSUM tile so exp runs as one
    ScalarE ACTIVATE per pair; P^T tiles are PV stationary operands,
    rhs = [V | ones]. Normalize with per-row reciprocal on VectorE.
    One AllToAll per batch reshards head-parallel -> token-parallel.
  - Phase C: exact output projection for this core's 256-token slice of
    each batch; b_proj enters via a rank-1 ones matmul.
  - Software-pipelined emission: B(b) interleaves one unit of A(b+1)
    (or C(b-3)) between each score group and its PV group, so the PE
    stream never waits on ScalarE exp or on a collective. DMA triggers
    live only on the sync/gpsimd queues; ScalarE does exp exclusively.

Self-contained: hardcodes all shapes; no file reads.
"""

import numpy as np
import ml_dtypes

import concourse.bacc as bacc
import concourse.tile as tile
import concourse.mybir as mybir
from concourse import bass_utils

BF16 = mybir.dt.bfloat16
F32 = mybir.dt.float32
AF = mybir.ActivationFunctionType

N_CORES = 8
B = 4
S = 2048
H = 2048
NH = 16
HD = 128
HPC = NH // N_CORES          # heads per core = 2
TOK = B * S                  # 8192
KCH = H // 128               # 16 hidden chunks
SC = 512                     # token chunk for projections / q-chunks
TPB_CH = S // SC             # 4 token chunks per batch
QB = S // 128                # 16 q/kv blocks per batch
SCALE = 1.0 / float(np.sqrt(HD))
VSTRIDE = 2 * (HD + 1)       # V storage: per tokblock [Vh0|1|Vh1|1]
TPB = S // N_CORES           # 256 tokens per core per batch after A2A

_CACHE: dict = {}
LAST_RESULT = None


def _build():
    nc = bacc.Bacc("TRN2", target_bir_lowering=False, debug=False,
                   num_devices=N_CORES)
    # Host-relayouted inputs: leading dim 128 = SBUF partition.
    xT = nc.dram_tensor("xT", [128, KCH, TOK], BF16, kind="ExternalInput")
    wqkv = nc.dram_tensor("wqkv", [128, KCH, 6 * HD], BF16,
                          kind="ExternalInput")
    wproj = nc.dram_tensor("wproj", [128, KCH, H], BF16,
                           kind="ExternalInput")
    bqkv = nc.dram_tensor("bqkv", [1, 6 * HD], BF16, kind="ExternalInput")
    bqk_t = nc.dram_tensor("bqk_t", [128, 4], F32, kind="ExternalInput")
    bproj = nc.dram_tensor("bproj", [1, H], BF16, kind="ExternalInput")
    mask = nc.dram_tensor("mask", [128, 128], BF16, kind="ExternalInput")
    out = nc.dram_tensor("out", [B * TPB, H], F32, kind="ExternalOutput")

    with tile.TileContext(nc) as tc:
        with (
            tc.tile_pool(name="const", bufs=1) as constp,
            tc.tile_pool(name="qkp", bufs=8) as qkp,
            tc.tile_pool(name="dram", bufs=1, space="DRAM") as dram,
            tc.tile_pool(name="xTp", bufs=3) as xTp,
            tc.tile_pool(name="psAC", bufs=2, space="PSUM") as psAC,
            tc.tile_pool(name="psS", bufs=2, space="PSUM") as psS,
            tc.tile_pool(name="psPV", bufs=2, space="PSUM") as psPV,
            tc.tile_pool(name="ptP", bufs=9) as ptP,
            tc.tile_pool(name="an4P", bufs=1) as an4P,
            tc.tile_pool(name="recP", bufs=4) as recP,
            tc.tile_pool(name="atP", bufs=4) as atP,
            tc.tile_pool(name="outP", bufs=1) as outP,
        ):
            # ---- resident weights / consts -------------------------------
            wq_t = constp.tile([128, KCH * 6 * HD], BF16, name="wq_t")
            nc.sync.dma_start(wq_t[:, 0:8 * 6 * HD], wqkv[:, 0:8, :])

            # First x chunk right behind the first weight half.
            xt_tiles = [None] * (2 * TPB_CH * B)   # (t, half) -> tile

            def load_x(t, half):
                xtile = xTp.tile([128, 8 * SC], BF16, name="xt")
                nc.sync.dma_start(
                    xtile[:],
                    xT[:, half * 8:(half + 1) * 8, t * SC:(t + 1) * SC])
                xt_tiles[2 * t + half] = xtile

            load_x(0, 0)
            nc.sync.dma_start(wq_t[:, 8 * 6 * HD:], wqkv[:, 8:16, :])
            load_x(0, 1)

            mask_sb = constp.tile([128, 128], BF16, name="mask_sb")
            nc.gpsimd.dma_start(mask_sb[:], mask[:])
            ones_sb = constp.tile([1, 128], BF16, name="ones_sb")
            nc.vector.memset(ones_sb[:], 1.0)
            bqkv_sb = constp.tile([1, 6 * HD], BF16, name="bqkv_sb")
            nc.gpsimd.dma_start(bqkv_sb[:], bqkv[:])
            bqkt_sb = constp.tile([128, 4], F32, name="bqkt_sb")
            nc.gpsimd.dma_start(bqkt_sb[:], bqk_t[:])
            bproj_sb = constp.tile([1, H], BF16, name="bproj_sb")
            nc.gpsimd.dma_start(bproj_sb[:], bproj[:])

            # V stores: 2 persistent slots; ones columns memset once.
            vst = [constp.tile([128, QB * VSTRIDE], BF16, name=f"vst{i}")
                   for i in range(2)]
            nc.vector.memset(vst[0][:], 1.0)
            nc.vector.memset(vst[1][:], 1.0)

            # W_proj resident, loaded late on gpsimd (first used in B3).
            wp_t = constp.tile([128, KCH * H], BF16, name="wp_t")

            def load_wproj(c):
                nc.gpsimd.dma_start(
                    wp_t[:, c * 4 * H:(c + 1) * 4 * H],
                    wproj[:, c * 4:(c + 1) * 4, :])

            qk_store = [None] * B
            a2a_in = [dram.tile([S, HPC * HD], BF16, name=f"cc_in{b}")
                      for b in range(B)]
            a2a_out = [dram.tile([S, HPC * HD], BF16, name=f"cc_out{b}")
                       for b in range(B)]
            at_w = [None] * B

            # ---- phase A as a unit generator -----------------------------
            def phase_a_units(b):
                """Yield 24 emission units for batch b's QKV projection."""
                qk_store[b] = [qkp.tile([128, S], BF16, name="qkt")
                               for _ in range(4)]
                vslot = vst[b % 2]
                for tloc in range(TPB_CH):
                    t = b * TPB_CH + tloc
                    for ob in range(4):      # q_h0, q_h1, k_h0, k_h1
                        def u_qk(t=t, tloc=tloc, ob=ob):
                            ps = psAC.tile([128, SC], F32, name="psa")
                            for kc in range(KCH):
                                xth = xt_tiles[2 * t + kc // 8]
                                nc.tensor.matmul(
                                    ps[:],
                                    wq_t[:, kc * 6 * HD + ob * 128:
                                         kc * 6 * HD + (ob + 1) * 128],
                                    xth[:, (kc % 8) * SC:(kc % 8 + 1) * SC],
                                    start=(kc == 0), stop=(kc == KCH - 1))
                            nc.vector.tensor_scalar_add(
                                qk_store[b][ob][:, tloc * SC:(tloc + 1) * SC],
                                ps[:], bqkt_sb[:, ob:ob + 1])
                        yield u_qk
                    for tbp in range(2):     # V blocks, natural layout
                        def u_v(t=t, tloc=tloc, tbp=tbp):
                            # prefetch next tchunk (crossing into the next
                            # batch at tloc==3; slot WAR throttles timing)
                            if t + 1 < B * TPB_CH:
                                load_x(t + 1, tbp)
                            for tb in (2 * tbp, 2 * tbp + 1):
                                psw = psAC.tile([128, SC], F32, name="psa")
                                ps = psw[:, 0:2 * HD]
                                for kc in range(KCH):
                                    xth = xt_tiles[2 * t + kc // 8]
                                    nc.tensor.matmul(
                                        ps,
                                        xth[:, (kc % 8) * SC + tb * 128:
                                            (kc % 8) * SC + (tb + 1) * 128],
                                        wq_t[:, kc * 6 * HD + 4 * HD:
                                             kc * 6 * HD + 6 * HD],
                                        start=(kc == 0), stop=False)
                                nc.tensor.matmul(ps, ones_sb[:],
                                                 bqkv_sb[:, 4 * HD:6 * HD],
                                                 start=False, stop=True)
                                base = (tloc * 4 + tb) * VSTRIDE
                                nc.vector.tensor_copy(
                                    vslot[:, base:base + HD], ps[:, 0:HD])
                                nc.vector.tensor_copy(
                                    vslot[:, base + HD + 1:
                                          base + 2 * HD + 1],
                                    ps[:, HD:2 * HD])
                        yield u_v

            # ---- phase C as a unit generator -----------------------------
            def phase_c_transposes(b):
                """Two wide DMA-transposes bring the received buffer in as
                a^T: partition = hidden-within-head-half, free =
                shard*TPB + token."""
                at_w[b] = []
                for half in range(2):
                    atile = atP.tile([128, S], BF16, name="at")
                    nc.sync.dma_start(
                        atile[:],
                        a2a_out[b][:, half * 128:(half + 1) * 128],
                        transpose=True)
                    at_w[b].append(atile)

            def phase_c_units(b):
                """Yield 8 units: output projection for this core's token
                slice of batch b."""
                for oc in range(4):
                    for tb in range(TPB // 128):
                        def u_c(oc=oc, tb=tb):
                            ps = psAC.tile([128, SC], F32, name="psa")
                            for hc in range(KCH):
                                nc.tensor.matmul(
                                    ps[:],
                                    at_w[b][hc % 2][:, (hc // 2) * TPB
                                                    + tb * 128:
                                                    (hc // 2) * TPB
                                                    + (tb + 1) * 128],
                                    wp_t[:, hc * H + oc * SC:
                                         hc * H + (oc + 1) * SC],
                                    start=(hc == 0), stop=False)
                            nc.tensor.matmul(
                                ps[:], ones_sb[:],
                                bproj_sb[:, oc * SC:(oc + 1) * SC],
                                start=False, stop=True)
                            ot = outP.tile([128, SC], F32, name="ot")
                            nc.vector.tensor_copy(ot[:], ps[:])
                            nc.gpsimd.dma_start(
                                out[b * TPB + tb * 128:
                                    b * TPB + (tb + 1) * 128,
                                    oc * SC:(oc + 1) * SC],
                                ot[:])
                        yield u_c

            # ---- phase B with interleaved filler units -------------------
            def phase_b(b, fillers):
                """Attention for batch b (both heads) + its AllToAll.
                Runs one filler unit between each score group and its PV
                group so the PE stream stays fed while ScalarE exps."""
                vslot = vst[b % 2]
                for h in range(HPC):
                    qt = qk_store[b][h]
                    kt = qk_store[b][2 + h]
                    for qc in range(4):
                        pts = []
                        for pr in range(2 * (qc + 1)):
                            ps2 = psS.tile([128, 2 * SC], F32, name="pss")
                            col0s = []
                            for j in range(2):
                                kb = 2 * pr + j
                                col0 = max(0, kb * 128 - qc * SC)
                                col0s.append(col0)
                                nc.tensor.matmul(
                                    ps2[:, j * SC + col0:(j + 1) * SC],
                                    kt[:, kb * 128:(kb + 1) * 128],
                                    qt[:, qc * SC + col0:(qc + 1) * SC],
                                    start=True, stop=True)
                            pt2 = ptP.tile([128, 2 * SC], BF16, name="pt")
                            if col0s[0] == col0s[1]:
                                # one ACT covers both written halves
                                nc.scalar.activation(
                                    pt2[:, col0s[0]:2 * SC],
                                    ps2[:, col0s[0]:2 * SC],
                                    AF.Exp, scale=SCALE)
                            else:
                                for j in range(2):
                                    nc.scalar.activation(
                                        pt2[:, j * SC + col0s[j]:(j + 1) * SC],
                                        ps2[:, j * SC + col0s[j]:(j + 1) * SC],
                                        AF.Exp, scale=SCALE)
                            for j in range(2):
                                kb = 2 * pr + j
                                if kb >= 4 * qc:
                                    col0 = max(0, kb * 128 - qc * SC)
                                    nc.vector.tensor_mul(
                                        pt2[:, j * SC + col0:
                                            j * SC + col0 + 128],
                                        pt2[:, j * SC + col0:
                                            j * SC + col0 + 128],
                                        mask_sb[:])
                            pts.append(pt2)
                        if fillers:
                            fillers.pop(0)()
                        an4 = an4P.tile([128, 4 * HD], BF16, name="an4")
                        for qb in range(4):
                            qg = qc * 4 + qb
                            po = psPV.tile([128, SC], F32,
                                           name="ppv")[:, 0:HD + 1]
                            for kb in range(qg + 1):
                                pr, j = divmod(kb, 2)
                                vbase = kb * VSTRIDE + h * (HD + 1)
                                nc.tensor.matmul(
                                    po[:],
                                    pts[pr][:, j * SC + qb * 128:
                                            j * SC + (qb + 1) * 128],
                                    vslot[:, vbase:vbase + HD + 1],
                                    start=(kb == 0), stop=(kb == qg))
                            rec = recP.tile([128, 1], F32, name="rec")
                            nc.vector.reciprocal(rec[:], po[:, HD:HD + 1])
                            nc.vector.tensor_scalar_mul(
                                an4[:, qb * 128:(qb + 1) * 128],
                                po[:, 0:HD], rec[:])
                        nc.sync.dma_start(
                            a2a_in[b][qc * SC:(qc + 1) * SC,
                                      h * HD:(h + 1) * HD]
                            .rearrange("(qb q) c -> q qb c", qb=4),
                            an4[:])
                nc.gpsimd.collective_compute(
                    "AllToAll",
                    mybir.AluOpType.bypass,
                    replica_groups=[list(range(N_CORES))],
                    ins=[a2a_in[b].opt()],
                    outs=[a2a_out[b].opt()],
                )

            def run_units(units):
                for u in units:
                    u()

            # ---- software-pipelined emission -----------------------------
            # A0(+wp spread) [B0*A1] A1rest [B1*A2] A2rest [B2*C0] A3
            # [B3*C1] C2 C3.  Each A2A gets >=1 C phase of PE cover; each
            # C(b)'s transposes are emitted well before first use; wproj
            # streams in during A0 so it never contends with a collective.
            a0 = list(phase_a_units(0))
            for i, u in enumerate(a0):
                u()
                if i % 6 == 5:
                    load_wproj(i // 6)
            a1 = list(phase_a_units(1))
            phase_b(0, a1[:8])
            run_units(a1[8:])
            a2 = list(phase_a_units(2))
            phase_b(1, a2[:8])
            run_units(a2[8:])
            phase_c_transposes(0)
            a3 = list(phase_a_units(3))
            phase_b(2, a3[:8])
            run_units(a3[8:])
            phase_c_transposes(1)
            c1 = list(phase_c_units(1))
            phase_b(3, c1)
            run_units(list(phase_c_units(0)))
            phase_c_transposes(2)
            phase_c_transposes(3)
            run_units(list(phase_c_units(2)))
            run_units(list(phase_c_units(3)))

    nc.compile()
    return nc


def _get_nc():
    if "nc" not in _CACHE:
        _CACHE["nc"] = _build()
    return _CACHE["nc"]


def kernel(hidden_states, W_attn, b_attn, W_proj, b_proj):
    global LAST_RESULT
    bf = ml_dtypes.bfloat16
    x = np.asarray(hidden_states, dtype=np.float32).reshape(TOK, H)
    xb = x.astype(bf)
    # [128, kc, t] layout: xT3[p, kc, t] = x[t, kc*128+p]
    xT3 = np.ascontiguousarray(
        xb.reshape(TOK, KCH, 128).transpose(2, 1, 0))
    Wa = np.asarray(W_attn, dtype=np.float32)
    ba = np.asarray(b_attn, dtype=np.float32)
    Wp = np.asarray(W_proj, dtype=np.float32).astype(bf)
    wp3 = np.ascontiguousarray(Wp.reshape(KCH, 128, H).transpose(1, 0, 2))
    bp = np.asarray(b_proj, dtype=np.float32).reshape(1, H).astype(bf)
    mask = np.triu(np.ones((128, 128), dtype=np.float32)).astype(bf)

    in_maps = []
    for c in range(N_CORES):
        h0 = c * HPC
        cols = []
        for part in range(3):          # q, k, v feature slices
            cols.append(np.arange(part * H + h0 * HD,
                                  part * H + (h0 + HPC) * HD))
        cols = np.concatenate(cols)    # 768 column indices
        wq = Wa[:, cols].astype(bf)
        wq3 = np.ascontiguousarray(
            wq.reshape(KCH, 128, 6 * HD).transpose(1, 0, 2))
        bq = ba[cols].reshape(1, 6 * HD).astype(bf)
        # per-partition bias for the 4 Q^T/K^T feature blocks
        bqk_t = np.ascontiguousarray(
            ba[cols[:4 * 128]].reshape(4, 128).T).astype(np.float32)
        in_maps.append({
            "xT": xT3,
            "wqkv": wq3,
            "wproj": wp3,
            "bqkv": bq,
            "bqk_t": bqk_t,
            "bproj": bp,
            "mask": mask,
        })

    nc = _get_nc()
    res = bass_utils.run_bass_kernel_spmd(
        nc, in_maps, core_ids=list(range(N_CORES)))
    LAST_RESULT = res

    full = np.empty((B, S, H), dtype=np.float32)
    for c in range(N_CORES):
        r = res.results[c]["out"]
        for b in range(B):
            full[b, c * TPB:(c + 1) * TPB, :] = r[b * TPB:(b + 1) * TPB, :]
    return full


# revision 23
# speedup vs baseline: 1.0454x; 1.0141x over previous
"""Causal multi-head attention (B=4, S=2048, H=2048, NH=16) on 8 TRN2 NeuronCores.

Strategy (tensor-parallel over heads + all-to-all reshard), v2:
  - Each core owns 2 heads. Host slices W_attn/b_attn per core, casts to
    bf16, and relayouts x^T / weights so every SBUF load is one big DMA
    ([128, kc, cols] layouts; fp32 accumulation happens in PSUM).
  - Phase A (per batch, 24 emission units): QKV projection from x^T
    tiles produces Q^T, K^T ([head_dim, tokens]) and V ([tokens,
    head_dim] with interleaved ones columns carrying a free softmax
    denominator through the PV matmul).
  - Phase B (per batch, per head): scores^T = K^T.T @ Q^T on causal
    blocks only, two k-blocks per 2-bank PSUM tile so exp runs as one
    ScalarE ACTIVATE per pair; P^T tiles are PV stationary operands,
    rhs = [V | ones]. Normalize with per-row reciprocal on VectorE.
    One AllToAll per batch reshards head-parallel -> token-parallel.
  - Phase C: exact output projection for this core's 256-token slice of
    each batch; b_proj enters via a rank-1 ones matmul.
  - Software-pipelined emission: B(b) interleaves one unit of A(b+1)
    (or C(b-3)) between each score group and its PV group, so the PE
    stream never waits on ScalarE exp or on a collective. DMA triggers
    live only on the sync/gpsimd queues; ScalarE does exp exclusively.

Self-contained: hardcodes all shapes; no file reads.
"""

import numpy as np
import ml_dtypes

import concourse.bacc as bacc
import concourse.tile as tile
import concourse.mybir as mybir
from concourse import bass_utils

BF16 = mybir.dt.bfloat16
F32 = mybir.dt.float32
AF = mybir.ActivationFunctionType

N_CORES = 8
B = 4
S = 2048
H = 2048
NH = 16
HD = 128
HPC = NH // N_CORES          # heads per core = 2
TOK = B * S                  # 8192
KCH = H // 128               # 16 hidden chunks
SC = 512                     # token chunk for projections / q-chunks
TPB_CH = S // SC             # 4 token chunks per batch
QB = S // 128                # 16 q/kv blocks per batch
SCALE = 1.0 / float(np.sqrt(HD))
VSTRIDE = 2 * (HD + 1)       # V storage: per tokblock [Vh0|1|Vh1|1]
TPB = S // N_CORES           # 256 tokens per core per batch after A2A

_CACHE: dict = {}
LAST_RESULT = None


def _build():
    nc = bacc.Bacc("TRN2", target_bir_lowering=False, debug=False,
                   num_devices=N_CORES)
    # Host-relayouted inputs: leading dim 128 = SBUF partition.
    xT = nc.dram_tensor("xT", [128, KCH, TOK], BF16, kind="ExternalInput")
    wqkv = nc.dram_tensor("wqkv", [128, KCH, 6 * HD], BF16,
                          kind="ExternalInput")
    wproj = nc.dram_tensor("wproj", [128, KCH, H], BF16,
                           kind="ExternalInput")
    bqkv = nc.dram_tensor("bqkv", [1, 6 * HD], BF16, kind="ExternalInput")
    bqk_t = nc.dram_tensor("bqk_t", [128, 4], F32, kind="ExternalInput")
    bproj = nc.dram_tensor("bproj", [1, H], BF16, kind="ExternalInput")
    mask = nc.dram_tensor("mask", [128, 128], BF16, kind="ExternalInput")
    out = nc.dram_tensor("out", [B * TPB, H], F32, kind="ExternalOutput")

    with tile.TileContext(nc) as tc:
        with (
            tc.tile_pool(name="const", bufs=1) as constp,
            tc.tile_pool(name="qkp", bufs=8) as qkp,
            tc.tile_pool(name="dram", bufs=1, space="DRAM") as dram,
            tc.tile_pool(name="xTp", bufs=7) as xTp,
            tc.tile_pool(name="psAC", bufs=2, space="PSUM") as psAC,
            tc.tile_pool(name="psS", bufs=2, space="PSUM") as psS,
            tc.tile_pool(name="psPV", bufs=2, space="PSUM") as psPV,
            tc.tile_pool(name="ptP", bufs=9) as ptP,
            tc.tile_pool(name="an4P", bufs=1) as an4P,
            tc.tile_pool(name="recP", bufs=4) as recP,
            tc.tile_pool(name="atP", bufs=4) as atP,
            tc.tile_pool(name="outP", bufs=1) as outP,
        ):
            # ---- resident weights / consts -------------------------------
            wq_t = constp.tile([128, KCH * 6 * HD], BF16, name="wq_t")
            nc.sync.dma_start(wq_t[:, 0:8 * 6 * HD], wqkv[:, 0:8, :])

            # First x chunk right behind the first weight half.
            xt_tiles = [None] * (4 * TPB_CH * B)   # (t, quarter) -> tile

            def load_x(t, q):
                xtile = xTp.tile([128, 4 * SC], BF16, name="xt")
                nc.sync.dma_start(
                    xtile[:],
                    xT[:, q * 4:(q + 1) * 4, t * SC:(t + 1) * SC])
                xt_tiles[4 * t + q] = xtile

            load_x(0, 0)
            load_x(0, 1)
            nc.sync.dma_start(wq_t[:, 8 * 6 * HD:], wqkv[:, 8:16, :])
            load_x(0, 2)
            load_x(0, 3)

            mask_sb = constp.tile([128, 128], BF16, name="mask_sb")
            nc.gpsimd.dma_start(mask_sb[:], mask[:])
            ones_sb = constp.tile([1, 128], BF16, name="ones_sb")
            nc.vector.memset(ones_sb[:], 1.0)
            bqkv_sb = constp.tile([1, 6 * HD], BF16, name="bqkv_sb")
            nc.gpsimd.dma_start(bqkv_sb[:], bqkv[:])
            bqkt_sb = constp.tile([128, 4], F32, name="bqkt_sb")
            nc.gpsimd.dma_start(bqkt_sb[:], bqk_t[:])
            bproj_sb = constp.tile([1, H], BF16, name="bproj_sb")
            nc.gpsimd.dma_start(bproj_sb[:], bproj[:])

            # V stores: 2 persistent slots; ones columns memset once.
            vst = [constp.tile([128, QB * VSTRIDE], BF16, name=f"vst{i}")
                   for i in range(2)]
            nc.vector.memset(vst[0][:], 1.0)
            nc.vector.memset(vst[1][:], 1.0)

            # W_proj resident, loaded late on gpsimd (first used in B3).
            wp_t = constp.tile([128, KCH * H], BF16, name="wp_t")

            def load_wproj(c):
                nc.gpsimd.dma_start(
                    wp_t[:, c * 4 * H:(c + 1) * 4 * H],
                    wproj[:, c * 4:(c + 1) * 4, :])

            qk_store = [None] * B
            a2a_in = [dram.tile([S, HPC * HD], BF16, name=f"cc_in{b}")
                      for b in range(B)]
            a2a_out = [dram.tile([S, HPC * HD], BF16, name=f"cc_out{b}")
                       for b in range(B)]
            at_w = [None] * B

            # ---- phase A as a unit generator -----------------------------
            def phase_a_units(b):
                """Yield 24 emission units for batch b's QKV projection."""
                qk_store[b] = [qkp.tile([128, S], BF16, name="qkt")
                               for _ in range(4)]
                vslot = vst[b % 2]
                for tloc in range(TPB_CH):
                    t = b * TPB_CH + tloc
                    for ob in range(4):      # q_h0, q_h1, k_h0, k_h1
                        def u_qk(t=t, tloc=tloc, ob=ob):
                            ps = psAC.tile([128, SC], F32, name="psa")
                            for kc in range(KCH):
                                xth = xt_tiles[4 * t + kc // 4]
                                nc.tensor.matmul(
                                    ps[:],
                                    wq_t[:, kc * 6 * HD + ob * 128:
                                         kc * 6 * HD + (ob + 1) * 128],
                                    xth[:, (kc % 4) * SC:(kc % 4 + 1) * SC],
                                    start=(kc == 0), stop=(kc == KCH - 1))
                            nc.vector.tensor_scalar_add(
                                qk_store[b][ob][:, tloc * SC:(tloc + 1) * SC],
                                ps[:], bqkt_sb[:, ob:ob + 1])
                        yield u_qk
                    for tbp in range(2):     # V blocks, natural layout
                        def u_v(t=t, tloc=tloc, tbp=tbp):
                            # prefetch next tchunk (crossing into the next
                            # batch at tloc==3; slot WAR throttles timing)
                            if t + 1 < B * TPB_CH:
                                load_x(t + 1, 2 * tbp)
                                load_x(t + 1, 2 * tbp + 1)
                            for tb in (2 * tbp, 2 * tbp + 1):
                                psw = psAC.tile([128, SC], F32, name="psa")
                                ps = psw[:, 0:2 * HD]
                                for kc in range(KCH):
                                    xth = xt_tiles[4 * t + kc // 4]
                                    nc.tensor.matmul(
                                        ps,
                                        xth[:, (kc % 4) * SC + tb * 128:
                                            (kc % 4) * SC + (tb + 1) * 128],
                                        wq_t[:, kc * 6 * HD + 4 * HD:
                                             kc * 6 * HD + 6 * HD],
                                        start=(kc == 0), stop=False)
                                nc.tensor.matmul(ps, ones_sb[:],
                                                 bqkv_sb[:, 4 * HD:6 * HD],
                                                 start=False, stop=True)
                                base = (tloc * 4 + tb) * VSTRIDE
                                nc.vector.tensor_copy(
                                    vslot[:, base:base + HD], ps[:, 0:HD])
                                nc.vector.tensor_copy(
                                    vslot[:, base + HD + 1:
                                          base + 2 * HD + 1],
                                    ps[:, HD:2 * HD])
                        yield u_v

            # ---- phase C as a unit generator -----------------------------
            def phase_c_transposes(b):
                """Two wide DMA-transposes bring the received buffer in as
                a^T: partition = hidden-within-head-half, free =
                shard*TPB + token."""
                at_w[b] = []
                for half in range(2):
                    atile = atP.tile([128, S], BF16, name="at")
                    nc.sync.dma_start(
                        atile[:],
                        a2a_out[b][:, half * 128:(half + 1) * 128],
                        transpose=True)
                    at_w[b].append(atile)

            def phase_c_units(b):
                """Yield 8 units: output projection for this core's token
                slice of batch b."""
                for oc in range(4):
                    for tb in range(TPB // 128):
                        def u_c(oc=oc, tb=tb):
                            ps = psAC.tile([128, SC], F32, name="psa")
                            for hc in range(KCH):
                                nc.tensor.matmul(
                                    ps[:],
                                    at_w[b][hc % 2][:, (hc // 2) * TPB
                                                    + tb * 128:
                                                    (hc // 2) * TPB
                                                    + (tb + 1) * 128],
                                    wp_t[:, hc * H + oc * SC:
                                         hc * H + (oc + 1) * SC],
                                    start=(hc == 0), stop=False)
                            nc.tensor.matmul(
                                ps[:], ones_sb[:],
                                bproj_sb[:, oc * SC:(oc + 1) * SC],
                                start=False, stop=True)
                            ot = outP.tile([128, SC], F32, name="ot")
                            nc.vector.tensor_copy(ot[:], ps[:])
                            nc.gpsimd.dma_start(
                                out[b * TPB + tb * 128:
                                    b * TPB + (tb + 1) * 128,
                                    oc * SC:(oc + 1) * SC],
                                ot[:])
                        yield u_c

            # ---- phase B with interleaved filler units -------------------
            def phase_b(b, fillers):
                """Attention for batch b (both heads) + its AllToAll.
                Runs one filler unit between each score group and its PV
                group so the PE stream stays fed while ScalarE exps."""
                vslot = vst[b % 2]
                for h in range(HPC):
                    qt = qk_store[b][h]
                    kt = qk_store[b][2 + h]
                    for qc in range(4):
                        pts = []
                        for pr in range(2 * (qc + 1)):
                            ps2 = psS.tile([128, 2 * SC], F32, name="pss")
                            col0s = []
                            for j in range(2):
                                kb = 2 * pr + j
                                col0 = max(0, kb * 128 - qc * SC)
                                col0s.append(col0)
                                nc.tensor.matmul(
                                    ps2[:, j * SC + col0:(j + 1) * SC],
                                    kt[:, kb * 128:(kb + 1) * 128],
                                    qt[:, qc * SC + col0:(qc + 1) * SC],
                                    start=True, stop=True)
                            pt2 = ptP.tile([128, 2 * SC], BF16, name="pt")
                            if col0s[0] == col0s[1]:
                                # one ACT covers both written halves
                                nc.scalar.activation(
                                    pt2[:, col0s[0]:2 * SC],
                                    ps2[:, col0s[0]:2 * SC],
                                    AF.Exp, scale=SCALE)
                            else:
                                for j in range(2):
                                    nc.scalar.activation(
                                        pt2[:, j * SC + col0s[j]:(j + 1) * SC],
                                        ps2[:, j * SC + col0s[j]:(j + 1) * SC],
                                        AF.Exp, scale=SCALE)
                            for j in range(2):
                                kb = 2 * pr + j
                                if kb >= 4 * qc:
                                    col0 = max(0, kb * 128 - qc * SC)
                                    nc.vector.tensor_mul(
                                        pt2[:, j * SC + col0:
                                            j * SC + col0 + 128],
                                        pt2[:, j * SC + col0:
                                            j * SC + col0 + 128],
                                        mask_sb[:])
                            pts.append(pt2)
                        if fillers:
                            fillers.pop(0)()
                        an4 = an4P.tile([128, 4 * HD], BF16, name="an4")
                        for qb in range(4):
                            qg = qc * 4 + qb
                            po = psPV.tile([128, SC], F32,
                                           name="ppv")[:, 0:HD + 1]
                            for kb in range(qg + 1):
                                pr, j = divmod(kb, 2)
                                vbase = kb * VSTRIDE + h * (HD + 1)
                                nc.tensor.matmul(
                                    po[:],
                                    pts[pr][:, j * SC + qb * 128:
                                            j * SC + (qb + 1) * 128],
                                    vslot[:, vbase:vbase + HD + 1],
                                    start=(kb == 0), stop=(kb == qg))
                            rec = recP.tile([128, 1], F32, name="rec")
                            nc.vector.reciprocal(rec[:], po[:, HD:HD + 1])
                            nc.vector.tensor_scalar_mul(
                                an4[:, qb * 128:(qb + 1) * 128],
                                po[:, 0:HD], rec[:])
                        nc.sync.dma_start(
                            a2a_in[b][qc * SC:(qc + 1) * SC,
                                      h * HD:(h + 1) * HD]
                            .rearrange("(qb q) c -> q qb c", qb=4),
                            an4[:])
                nc.gpsimd.collective_compute(
                    "AllToAll",
                    mybir.AluOpType.bypass,
                    replica_groups=[list(range(N_CORES))],
                    ins=[a2a_in[b].opt()],
                    outs=[a2a_out[b].opt()],
                )

            def run_units(units):
                for u in units:
                    u()

            # ---- software-pipelined emission -----------------------------
            # A0(+wp spread) [B0*A1] A1rest [B1*A2] A2rest [B2*C0] A3
            # [B3*C1] C2 C3.  Each A2A gets >=1 C phase of PE cover; each
            # C(b)'s transposes are emitted well before first use; wproj
            # streams in during A0 so it never contends with a collective.
            a0 = list(phase_a_units(0))
            for i, u in enumerate(a0):
                u()
                if i % 6 == 5:
                    load_wproj(i // 6)
            a1 = list(phase_a_units(1))
            phase_b(0, a1[:8])
            run_units(a1[8:])
            a2 = list(phase_a_units(2))
            phase_b(1, a2[:8])
            run_units(a2[8:])
            phase_c_transposes(0)
            a3 = list(phase_a_units(3))
            phase_b(2, a3[:8])
            run_units(a3[8:])
            phase_c_transposes(1)
            c1 = list(phase_c_units(1))
            phase_b(3, c1)
            run_units(list(phase_c_units(0)))
            phase_c_transposes(2)
            phase_c_transposes(3)
            run_units(list(phase_c_units(2)))
            run_units(list(phase_c_units(3)))

    nc.compile()
    return nc


def _get_nc():
    if "nc" not in _CACHE:
        _CACHE["nc"] = _build()
    return _CACHE["nc"]


def kernel(hidden_states, W_attn, b_attn, W_proj, b_proj):
    global LAST_RESULT
    bf = ml_dtypes.bfloat16
    x = np.asarray(hidden_states, dtype=np.float32).reshape(TOK, H)
    xb = x.astype(bf)
    # [128, kc, t] layout: xT3[p, kc, t] = x[t, kc*128+p]
    xT3 = np.ascontiguousarray(
        xb.reshape(TOK, KCH, 128).transpose(2, 1, 0))
    Wa = np.asarray(W_attn, dtype=np.float32)
    ba = np.asarray(b_attn, dtype=np.float32)
    Wp = np.asarray(W_proj, dtype=np.float32).astype(bf)
    wp3 = np.ascontiguousarray(Wp.reshape(KCH, 128, H).transpose(1, 0, 2))
    bp = np.asarray(b_proj, dtype=np.float32).reshape(1, H).astype(bf)
    mask = np.triu(np.ones((128, 128), dtype=np.float32)).astype(bf)

    in_maps = []
    for c in range(N_CORES):
        h0 = c * HPC
        cols = []
        for part in range(3):          # q, k, v feature slices
            cols.append(np.arange(part * H + h0 * HD,
                                  part * H + (h0 + HPC) * HD))
        cols = np.concatenate(cols)    # 768 column indices
        wq = Wa[:, cols].astype(bf)
        wq3 = np.ascontiguousarray(
            wq.reshape(KCH, 128, 6 * HD).transpose(1, 0, 2))
        bq = ba[cols].reshape(1, 6 * HD).astype(bf)
        # per-partition bias for the 4 Q^T/K^T feature blocks
        bqk_t = np.ascontiguousarray(
            ba[cols[:4 * 128]].reshape(4, 128).T).astype(np.float32)
        in_maps.append({
            "xT": xT3,
            "wqkv": wq3,
            "wproj": wp3,
            "bqkv": bq,
            "bqk_t": bqk_t,
            "bproj": bp,
            "mask": mask,
        })

    nc = _get_nc()
    res = bass_utils.run_bass_kernel_spmd(
        nc, in_maps, core_ids=list(range(N_CORES)))
    LAST_RESULT = res

    full = np.empty((B, S, H), dtype=np.float32)
    for c in range(N_CORES):
        r = res.results[c]["out"]
        for b in range(B):
            full[b, c * TPB:(c + 1) * TPB, :] = r[b * TPB:(b + 1) * TPB, :]
    return full
